# revision 1
# baseline (speedup 1.0000x reference)
"""Trainium2 Bass kernel for nn_AdaptiveMultiBoxLoss (SSD multibox distillation loss).

Data-parallel over the batch dim across 8 NeuronCores.  Each core computes
partial sums (smooth-L1 loc losses, CE conf losses with hard-negative mining
via a per-row binary-search threshold top-k) over its 8 batch rows; the host
sums the 8x16 partials and performs the final division by N.

Key device-side decompositions:
  loss_c = sum_pos(lse) - sum_all conf[p, ct_p] + sum_all conf[:,0]
           - sum_pos conf[:,0] + topk(lc_masked)
  (exploits that ~98% of priors are background so the CE gather is column 0;
   the true gather sum is a one-hot trace accumulated on the TensorEngine)
  topk per row: binary-search a threshold with exact counts
  (tensor_scalar is_gt + fused accumulate), then an exact correction pass.
"""

import os
import sys

sys.path.insert(0, "/opt/trn_rl_repo")

from contextlib import ExitStack

import numpy as np

import concourse.bass as bass
import concourse.bacc as bacc
import concourse.mybir as mybir
import concourse.tile as tile
from concourse.bass_utils import run_bass_kernel_spmd

F32 = mybir.dt.float32
BF16 = mybir.dt.bfloat16
I32 = mybir.dt.int32
ALU = mybir.AluOpType
ACT = mybir.ActivationFunctionType

# ---- problem geometry (hardcoded) ----
B, P, C = 64, 8732, 81
NCORES = 8
R = B // NCORES            # 8 batch rows per core
NT = 69                    # 128-prior tiles per row (68 full + 1x28)
TFULL, TREM = 68, 28
TCOL = R * NT              # 552 columns in row-tiled layout
NFB, FBT = 3, 23           # conf stream: 3 blocks/row x 23 tiles
FBF = FBT * C              # 1863
LTT, LTFULL, LTREM = 546, 545, 96   # loc flat tiling: 546 tiles of 128 rows
LF = LTT * 4               # 2184
NPART = 16
NE_CONST = 128 * LF        # every element of the padded loc tile contributes +1
NITER = 7                  # binary search iterations (2*lc domain)
HI_INIT = 32.0

# partials columns
(COL_BT, COL_BS, COL_AT, COL_CT, COL_DT, COL_AS, COL_CS, COL_DS,
 COL_LT, COL_LS, COL_TKT, COL_TKS, COL_NP) = range(13)

STAGE = int(os.environ.get("K_STAGE", "9"))


def build_nc():
    nc = bacc.Bacc("TRN2", target_bir_lowering=False, debug=False,
                   num_devices=NCORES)

    conf_T = nc.declare_dram_parameter("conf_T", [R, P, C], F32, isOutput=False)
    conf_S = nc.declare_dram_parameter("conf_S", [R, P, C], F32, isOutput=False)
    loc_T = nc.declare_dram_parameter("loc_T", [128 * LTT, 4], F32, isOutput=False)
    loc_S = nc.declare_dram_parameter("loc_S", [128 * LTT, 4], F32, isOutput=False)
    loc_t = nc.declare_dram_parameter("loc_t", [128 * LTT, 4], F32, isOutput=False)
    ctp = nc.declare_dram_parameter("ctp", [128 * LTT], I32, isOutput=False)
    conf_t = nc.declare_dram_parameter("conf_t", [R, P], I32, isOutput=False)
    iota_p = nc.declare_dram_parameter("iota", [128, FBF], F32, isOutput=False)
    onesw_p = nc.declare_dram_parameter("ones8w", [8, 128], F32, isOutput=False)
    eye_p = nc.declare_dram_parameter("eye81", [81, 81], F32, isOutput=False)
    ones_p = nc.declare_dram_parameter("ones128", [128, 1], F32, isOutput=False)
    out_p = nc.declare_dram_parameter("out", [1, NPART], F32, isOutput=True)

    with tile.TileContext(nc) as tc, ExitStack() as ctx:
        cpool = ctx.enter_context(tc.tile_pool(name="consts", bufs=1))
        pers = ctx.enter_context(tc.tile_pool(name="pers", bufs=1))
        small = ctx.enter_context(tc.tile_pool(name="small", bufs=1))
        pool_c = ctx.enter_context(tc.tile_pool(name="conf", bufs=3))
        pool_eT = ctx.enter_context(tc.tile_pool(name="expT", bufs=2))
        pool_eS = ctx.enter_context(tc.tile_pool(name="expS", bufs=2))
        pool_eq = ctx.enter_context(tc.tile_pool(name="eq", bufs=2))
        psum = ctx.enter_context(tc.tile_pool(name="ps", bufs=4, space="PSUM"))
        pstr = ctx.enter_context(tc.tile_pool(name="tr", bufs=1, space="PSUM"))

        # ---- constants ----
        iota_sb = cpool.tile([128, FBF], F32)
        onesw_sb = cpool.tile([8, 128], F32)
        eye_sb = cpool.tile([81, 81], F32)
        ones_sb = cpool.tile([128, 1], F32)
        nc.sync.dma_start(out=iota_sb[:, :], in_=iota_p.ap())
        nc.sync.dma_start(out=onesw_sb[:, :], in_=onesw_p.ap())
        nc.sync.dma_start(out=eye_sb[:, :], in_=eye_p.ap())
        nc.sync.dma_start(out=ones_sb[:, :], in_=ones_p.ap())

        # ---- persistent tensors ----
        ctf_i = pers.tile([128, TCOL], I32)
        ctf = pers.tile([128, TCOL], F32)
        posf = pers.tile([128, TCOL], F32)
        ominus = pers.tile([128, TCOL], F32)
        vmask = pers.tile([128, TCOL], F32)
        sumexp = {x: pers.tile([128, TCOL], F32, name=f"sumexp{x}") for x in "TS"}
        conf0 = {x: pers.tile([128, TCOL], F32, name=f"conf0{x}") for x in "TS"}
        lse = {x: pers.tile([128, TCOL], F32, name=f"lse{x}") for x in "TS"}
        lcm = {x: pers.tile([128, TCOL], F32, name=f"lcm{x}") for x in "TS"}
        partials = pers.tile([128, NPART], F32)
        sgnjunk = pers.tile([128, TCOL], F32)
        sjunk = {x: pers.tile([128, TCOL], F32, name=f"sjunk{x}") for x in "TS"}

        ctfl_i = pers.tile([128, LTT], I32)
        ctfl = pers.tile([128, LTT], F32)
        posml = pers.tile([128, LTT], F32)
        locsb = {n: pers.tile([128, 2, 1096], F32, name=f"loc{n}")
                 for n in ("T", "S", "t")}
        ld = pers.tile([128, LF], F32)
        lu = pers.tile([128, LF], BF16)
        lc_ = pers.tile([128, LF], BF16)
        lm = pers.tile([128, LF], BF16)

        nc.gpsimd.memset(partials[:, :], 0.0)

        # ---- conf_t: row-tiled layout, partition-major within each row ----
        # column r*NT + fb*FBT + j on partition p holds prior 69*p + fb*23 + j
        for r in range(R):
            nc.gpsimd.memset(ctf_i[96:128, r * NT:(r + 1) * NT], -1)
            nc.sync.dma_start(
                out=ctf_i[0:126, r * NT:(r + 1) * NT],
                in_=conf_t.ap()[r, 0:126 * NT].rearrange("(p t) -> p t", t=NT))
            nc.sync.dma_start(
                out=ctf_i[126:127, r * NT:r * NT + 38],
                in_=conf_t.ap()[r, 126 * NT:P].unsqueeze(0))
        nc.vector.tensor_copy(out=ctf[:, :], in_=ctf_i[:, :])
        nc.vector.tensor_scalar(out=posf[:, :], in0=ctf[:, :], scalar1=0.5,
                                scalar2=None, op0=ALU.is_gt)
        nc.vector.tensor_scalar(out=vmask[:, :], in0=ctf[:, :], scalar1=-0.5,
                                scalar2=2.0, op0=ALU.is_gt, op1=ALU.mult)
        nc.vector.scalar_tensor_tensor(out=ominus[:, :], in0=posf[:, :],
                                       scalar=-2.0, in1=vmask[:, :],
                                       op0=ALU.mult, op1=ALU.add)

        # num_pos per row -> k
        npp = small.tile([128, 8], F32)
        nc.vector.tensor_reduce(out=npp[:, :],
                                in_=posf[:, :].rearrange("p (r t) -> p r t", r=R),
                                axis=mybir.AxisListType.X, op=ALU.add)
        ps_np = psum.tile([8, 1], F32, tag="ps")
        nc.tensor.matmul(ps_np[:, :], lhsT=npp[:, :], rhs=ones_sb[:, :],
                         start=True, stop=True)
        np8 = small.tile([8, 1], F32)
        nc.vector.tensor_copy(out=np8[:, :], in_=ps_np[:, :])
        k8 = small.tile([8, 1], F32)
        nc.vector.tensor_scalar(out=k8[:, :], in0=np8[:, :], scalar1=3.0,
                                scalar2=float(P - 1), op0=ALU.mult, op1=ALU.min)
        nc.vector.tensor_copy(out=partials[0:8, COL_NP:COL_NP + 1], in_=np8[:, :])

        # ---- conf streaming loop ----
        pstr2 = pstr.tile([81, 2 * C], F32)
        if STAGE >= 3:
            nmm = 0
            fbidx = [0]
            total_mm = R * NFB * FBT
            for r in range(R):
                for fb in range(NFB):
                    cb = r * NT + fb * FBT
                    ctile = pool_c.tile([128, 2, FBT, C], F32, name="ctile")
                    ex_fb = {"T": pool_eT.tile([128, FBT, C], BF16, name="exT"),
                             "S": pool_eS.tile([128, FBT, C], BF16, name="exS")}
                    if fbidx[0] < 3:
                        # fresh SBUF slot: clear pad partitions once so
                        # later exp() of unwritten pads stays finite
                        nc.scalar.memzero(ctile[96:128, :, :, :])
                    for xi, (x, param) in enumerate((("T", conf_T),
                                                     ("S", conf_S))):
                        t = ctile[:, xi, :, :]
                        # src view: conf[r, 69p + fb*23 + j, c]
                        rowv = param.ap()[r, :, :]
                        main = rowv[0:126 * NT, :].rearrange(
                            "(p t) c -> p t c", t=NT)
                        nc.sync.dma_start(
                            out=t[0:126, :, :],
                            in_=main[:, fb * FBT:(fb + 1) * FBT, :])
                        if fb == 0:
                            nc.sync.dma_start(
                                out=t[126:127, :, :],
                                in_=rowv[126 * NT:126 * NT + FBT, :]
                                    .unsqueeze(0))
                        elif fb == 1:
                            nc.sync.dma_start(
                                out=t[126:127, 0:15, :],
                                in_=rowv[126 * NT + FBT:P, :].unsqueeze(0))
                        nc.scalar.activation(out=ex_fb[x][:, :, :],
                                             in_=t[:, :, :], func=ACT.Exp)

                    eq_t = pool_eq.tile([128, FBT, C], F32, name="eqt")
                    ctb_view = ctf[:, cb:cb + FBT].unsqueeze(2).broadcast_to(
                        (128, FBT, C))
                    nc.vector.tensor_tensor(
                        out=eq_t[:, :, :],
                        in0=iota_sb[:, :].rearrange("p (t c) -> p t c", c=C),
                        in1=ctb_view, op=ALU.is_equal)

                    for xi, x in enumerate("TS"):
                        e1 = pool_eq.tile([128, FBT, 40], BF16, name="e1",
                                          tag="e1")
                        nc.vector.tensor_tensor(out=e1[:, :, :],
                                                in0=ex_fb[x][:, :, 0:40],
                                                in1=ex_fb[x][:, :, 40:80],
                                                op=ALU.add)
                        nc.vector.tensor_reduce(out=sumexp[x][:, cb:cb + FBT],
                                                in_=e1[:, :, :],
                                                axis=mybir.AxisListType.X,
                                                op=ALU.add)
                        nc.vector.tensor_tensor(out=sumexp[x][:, cb:cb + FBT],
                                                in0=sumexp[x][:, cb:cb + FBT],
                                                in1=ex_fb[x][:, :, 80],
                                                op=ALU.add)
                        nc.vector.tensor_copy(out=conf0[x][:, cb:cb + FBT],
                                              in_=ctile[:, xi, :, 0])
                    # one-hot trace for both tensors in one matmul:
                    # psum[m, (xi, c)] += sum_p eq[p, m] * conf[p, xi, t, c]
                    for t in range(FBT):
                        nc.tensor.matmul(
                            pstr2[:, :].rearrange("m (x c) -> m x c", x=2),
                            lhsT=eq_t[:, t, :],
                            rhs=ctile[:, :, t, :],
                            start=(nmm == 0), stop=(nmm == total_mm - 1))
                        nmm += 1
                    fbidx[0] += 1
                # per-row tail once the row's 3 blocks are done
                if STAGE >= 4:
                    rc = r * NT
                    for x in "TS":
                        nc.scalar.activation(out=lse[x][:, rc:rc + NT],
                                             in_=sumexp[x][:, rc:rc + NT],
                                             func=ACT.Ln)
                        nc.vector.scalar_tensor_tensor(
                            out=sumexp[x][:, rc:rc + NT],
                            in0=conf0[x][:, rc:rc + NT], scalar=-1.0,
                            in1=lse[x][:, rc:rc + NT],
                            op0=ALU.mult, op1=ALU.add)
                        nc.vector.tensor_tensor(out=lcm[x][:, rc:rc + NT],
                                                in0=sumexp[x][:, rc:rc + NT],
                                                in1=ominus[:, rc:rc + NT],
                                                op=ALU.mult)

        # ---- per-tensor epilogue: global partial sums ----
        if STAGE >= 4:
            # B trace extraction for both tensors at once
            eyev = eye_sb[:, :].unsqueeze(1).broadcast_to((81, 2, C))
            nc.vector.tensor_tensor(
                out=sgnjunk[0:81, 0:2 * C].rearrange("m (x c) -> m x c", x=2),
                in0=pstr2[:, :].rearrange("m (x c) -> m x c", x=2),
                in1=eyev, op=ALU.mult)
            nc.vector.tensor_reduce(
                out=partials[0:81, COL_BT:COL_BS + 1],
                in_=sgnjunk[0:81, 0:2 * C].rearrange("m (x c) -> m x c", x=2),
                axis=mybir.AxisListType.X, op=ALU.add)
            for x, (colA, colCc, colD) in (
                    ("T", (COL_AT, COL_CT, COL_DT)),
                    ("S", (COL_AS, COL_CS, COL_DS))):
                # A = sum(lse * posf)
                nc.vector.tensor_tensor(out=sgnjunk[:, :], in0=lse[x][:, :],
                                        in1=posf[:, :], op=ALU.mult)
                nc.vector.tensor_reduce(out=partials[:, colA:colA + 1],
                                        in_=sgnjunk[:, :],
                                        axis=mybir.AxisListType.X, op=ALU.add)
                # C2 = sum conf0 * vmask2  (= 2*C, pads excluded)
                nc.vector.tensor_tensor(out=sgnjunk[:, :], in0=conf0[x][:, :],
                                        in1=vmask[:, :], op=ALU.mult)
                nc.vector.tensor_reduce(out=partials[:, colCc:colCc + 1],
                                        in_=sgnjunk[:, :],
                                        axis=mybir.AxisListType.X, op=ALU.add)
                # D = sum conf0 * posf
                nc.vector.tensor_tensor(out=sgnjunk[:, :], in0=conf0[x][:, :],
                                        in1=posf[:, :], op=ALU.mult)
                nc.vector.tensor_reduce(out=partials[:, colD:colD + 1],
                                        in_=sgnjunk[:, :],
                                        axis=mybir.AxisListType.X, op=ALU.add)

        # ---- binary search for per-row top-k count thresholds ----
        # natural layout: per-row thresholds broadcast to [128, 8] via
        # diag(t) matmul, counts via 8 per-row tensor_scalar+accum ops.
        lo = {x: small.tile([8, 1], F32, name=f"lo{x}") for x in "TS"}
        hi = {x: small.tile([8, 1], F32, name=f"hi{x}") for x in "TS"}
        tmid = {x: small.tile([8, 1], F32, name=f"tm{x}") for x in "TS"}
        ge = {x: small.tile([8, 1], I32, name=f"ge{x}") for x in "TS"}
        gei = {x: small.tile([8, 1], I32, name=f"gei{x}") for x in "TS"}
        s8 = {x: small.tile([8, 1], F32, name=f"s8{x}") for x in "TS"}
        diag8 = {x: small.tile([8, 8], F32, name=f"dg{x}") for x in "TS"}
        trep = {x: small.tile([128, 8], F32, name=f"trep{x}") for x in "TS"}
        cnt8 = {x: small.tile([128, 8], F32, name=f"cnt8{x}") for x in "TS"}
        s8p = {x: small.tile([128, 8], F32, name=f"s8p{x}") for x in "TS"}
        ns8 = {x: small.tile([8, 2], F32, name=f"ns8{x}") for x in "TS"}
        tk = {x: small.tile([8, 1], F32, name=f"tk{x}") for x in "TS"}

        def bcast_rows(vec8, x):
            # trep[q, r] = vec8[r]  for all partitions q
            nc.vector.tensor_tensor(out=diag8[x][:, :], in0=eye_sb[0:8, 0:8],
                                    in1=vec8[:, :].broadcast_to((8, 8)),
                                    op=ALU.mult)
            psA = psum.tile([128, 8], F32, name="psA", tag="ps")
            nc.tensor.matmul(psA[:, :], lhsT=onesw_sb[:, :],
                             rhs=diag8[x][:, :], start=True, stop=True)
            nc.vector.tensor_copy(out=trep[x][:, :], in_=psA[:, :])

        def row_counts(x, src_tile, out128x8):
            trv = trep[x][:, :].unsqueeze(2).broadcast_to((128, R, NT))
            nc.vector.tensor_tensor(
                out=sjunk[x][:, :].rearrange("p (r t) -> p r t", r=R),
                in0=src_tile[:, :].rearrange("p (r t) -> p r t", r=R),
                in1=trv, op=ALU.is_gt)
            nc.vector.tensor_reduce(
                out=out128x8[:, :],
                in_=sjunk[x][:, :].rearrange("p (r t) -> p r t", r=R),
                axis=mybir.AxisListType.X, op=ALU.add)

        if STAGE >= 5:
            for x in "TS":
                nc.gpsimd.memset(lo[x][:, :], 0.0)
                nc.gpsimd.memset(hi[x][:, :], HI_INIT)
            for it in range(NITER):
                for x in "TS":
                    nc.vector.tensor_tensor(out=tmid[x][:, :], in0=lo[x][:, :],
                                            in1=hi[x][:, :], op=ALU.add)
                    nc.vector.tensor_scalar(out=tmid[x][:, :], in0=tmid[x][:, :],
                                            scalar1=0.5, scalar2=None,
                                            op0=ALU.mult)
                    bcast_rows(tmid[x], x)
                    row_counts(x, lcm[x], cnt8[x])
                    psB = psum.tile([8, 1], F32, name="psB", tag="ps")
                    nc.tensor.matmul(psB[:, :], lhsT=cnt8[x][:, :],
                                     rhs=ones_sb[:, :], start=True, stop=True)
                    nc.vector.tensor_copy(out=s8[x][:, :], in_=psB[:, :])
                    nc.vector.tensor_tensor(out=ge[x][:, :], in0=s8[x][:, :],
                                            in1=k8[:, :], op=ALU.is_ge)
                    nc.vector.copy_predicated(out=lo[x][:, :], mask=ge[x][:, :],
                                              data=tmid[x][:, :])
                    nc.vector.tensor_scalar(out=gei[x][:, :], in0=ge[x][:, :],
                                            scalar1=1, scalar2=None,
                                            op0=ALU.bitwise_xor)
                    nc.vector.copy_predicated(out=hi[x][:, :], mask=gei[x][:, :],
                                              data=tmid[x][:, :])

        # ---- exact pass at t* = lo ----
        if STAGE >= 6:
            for x, colk in (("T", COL_TKT), ("S", COL_TKS)):
                bcast_rows(lo[x], x)
                row_counts(x, lcm[x], cnt8[x])
                nc.vector.tensor_tensor(out=lse[x][:, :], in0=lcm[x][:, :],
                                        in1=sjunk[x][:, :], op=ALU.mult)
                nc.vector.tensor_reduce(
                    out=s8p[x][:, :],
                    in_=lse[x][:, :].rearrange("p (r t) -> p r t", r=R),
                    axis=mybir.AxisListType.X, op=ALU.add)
                psC = psum.tile([8, 2], F32, name="psC", tag="ps")
                nc.tensor.matmul(psC[:, 0:1], lhsT=cnt8[x][:, :],
                                 rhs=ones_sb[:, :], start=True, stop=True)
                nc.tensor.matmul(psC[:, 1:2], lhsT=s8p[x][:, :],
                                 rhs=ones_sb[:, :], start=True, stop=True)
                nc.vector.tensor_copy(out=ns8[x][:, :], in_=psC[:, :])
                # topk = S* + (k - n*) * t*
                nc.vector.tensor_tensor(out=tk[x][:, :], in0=k8[:, :],
                                        in1=ns8[x][:, 0:1], op=ALU.subtract)
                nc.vector.tensor_tensor(out=tk[x][:, :], in0=tk[x][:, :],
                                        in1=lo[x][:, :], op=ALU.mult)
                nc.vector.tensor_tensor(out=tk[x][:, :], in0=tk[x][:, :],
                                        in1=ns8[x][:, 1:2], op=ALU.add)
                nc.vector.tensor_scalar(out=tk[x][:, :], in0=tk[x][:, :],
                                        scalar1=0.5, scalar2=None, op0=ALU.mult)
                nc.vector.tensor_copy(out=partials[0:8, colk:colk + 1],
                                      in_=tk[x][:, :])

        # ---- conf_t flat layout (for loc masking) ----
        # gates: loc DMAs wait for end-of-streaming; the loc vector chain
        # waits for the search to finish so it fills the tail instead of
        # stalling the vector FIFO mid-kernel.
        if STAGE >= 6:
            for n in ("T", "S", "t"):
                nc.vector.tensor_copy(out=locsb[n][0:8, 0, 0:1],
                                      in_=lcm["S"][0:8, TCOL - 1:TCOL])
            nc.vector.tensor_copy(out=ld[0:8, 0:1], in_=tk["S"][:, :])
        nc.sync.dma_start(
            out=ctfl_i[:, :],
            in_=ctp.ap().rearrange("(p t) -> p t", t=LTT))
        nc.vector.tensor_copy(out=ctfl[:, :], in_=ctfl_i[:, :])
        nc.vector.tensor_scalar(out=posml[:, :], in0=ctfl[:, :], scalar1=0.5,
                                scalar2=None, op0=ALU.is_gt)

        # ---- loc DMAs ----
        for name, param in (("T", loc_T), ("S", loc_S), ("t", loc_t)):
            dst = locsb[name]
            flat = param.ap()
            for a in range(2):
                nc.sync.dma_start(
                    out=dst[:, a, 0:1092],
                    in_=flat[:, :]
                        .rearrange("(p a j) f -> p a (j f)", a=2, j=273)[:, a, :])

        # ---- loc smooth-L1 (masked, sum) ----
        # per element: 0.5*min(u,1)^2 + max(u,1) - 1 with u = |loc - loc_t|*pos
        # masked/pad elements contribute exactly +1, subtracted as NE_CONST.
        if STAGE >= 2:
            posml4 = (posml[:, :].rearrange("p (a j) -> p a j", a=2)
                      .unsqueeze(3).broadcast_to((128, 2, 273, 4)))
            for x, col in (("T", COL_LT), ("S", COL_LS)):
                nc.vector.tensor_tensor(
                    out=ld[:, :].rearrange("p (a e) -> p a e", a=2),
                    in0=locsb[x][:, :, 0:1092],
                    in1=locsb["t"][:, :, 0:1092],
                    op=ALU.subtract)
                nc.vector.tensor_tensor(
                    out=lu[:, :].rearrange("p (a j f) -> p a j f", a=2, j=273),
                    in0=ld[:, :].rearrange("p (a j f) -> p a j f", a=2, j=273),
                    in1=posml4, op=ALU.mult)
                nc.scalar.activation(out=lu[:, :], in_=lu[:, :], func=ACT.Abs)
                nc.vector.tensor_scalar(out=lc_[:, :], in0=lu[:, :], scalar1=1.0,
                                        scalar2=None, op0=ALU.min)
                nc.vector.tensor_scalar(out=lm[:, :], in0=lu[:, :], scalar1=1.0,
                                        scalar2=None, op0=ALU.max)
                nc.scalar.activation(out=lc_[:, :], in_=lc_[:, :], func=ACT.Square,
                                     scale=float(1.0 / np.sqrt(2.0)))
                nc.vector.tensor_tensor(out=ld[:, :], in0=lc_[:, :],
                                        in1=lm[:, :], op=ALU.add)
                nc.vector.tensor_reduce(out=partials[:, col:col + 1],
                                        in_=ld[:, :],
                                        axis=mybir.AxisListType.X, op=ALU.add)


        # ---- final partition reduce of partials -> out ----
        psF = psum.tile([1, NPART], F32, name="psF", tag="ps")
        nc.tensor.matmul(psF[:, :], lhsT=ones_sb[:, :], rhs=partials[:, :],
                         start=True, stop=True)
        fin = small.tile([1, NPART], F32)
        nc.vector.tensor_copy(out=fin[:, :], in_=psF[:, :])
        nc.sync.dma_start(out=out_p.ap(), in_=fin[:, :])
    nc.finalize()
    return nc


_NC_CACHE = None


def _get_nc():
    global _NC_CACHE
    if _NC_CACHE is None:
        _NC_CACHE = build_nc()
    return _NC_CACHE


def _host_consts():
    iota = np.ascontiguousarray(
        np.tile(np.arange(C, dtype=np.float32), FBT)[None, :].repeat(128, 0))
    ones8w = np.ones((8, 128), np.float32)
    eye81 = np.eye(81, dtype=np.float32)
    ones = np.ones((128, 1), np.float32)
    return iota, ones8w, eye81, ones


def _build_in_maps(inputs):
    conf_T = np.ascontiguousarray(np.asarray(inputs["conf_dataT"], np.float32))
    conf_S = np.ascontiguousarray(np.asarray(inputs["conf_dataS"], np.float32))
    loc_T = np.ascontiguousarray(np.asarray(inputs["loc_dataT"], np.float32))
    loc_S = np.ascontiguousarray(np.asarray(inputs["loc_dataS"], np.float32))
    loc_t = np.ascontiguousarray(np.asarray(inputs["loc_t"], np.float32))
    ct = np.ascontiguousarray(np.asarray(inputs["conf_t"], np.int32))
    PADN = 128 * LTT - R * P

    def _padloc(a):
        flat = a.reshape(R * P, 4)
        return np.ascontiguousarray(
            np.pad(flat, ((0, PADN), (0, 0))))
    iota, ones8w, eye81, ones = _host_consts()
    in_maps = []
    for d in range(NCORES):
        sl = slice(d * R, (d + 1) * R)
        ctsl = ct[sl]
        in_maps.append({
            "conf_T": conf_T[sl], "conf_S": conf_S[sl],
            "loc_T": _padloc(loc_T[sl]), "loc_S": _padloc(loc_S[sl]),
            "loc_t": _padloc(loc_t[sl]),
            "conf_t": ctsl,
            "ctp": np.ascontiguousarray(
                np.pad(ctsl.ravel(), (0, PADN), constant_values=-1)),
            "iota": iota, "ones8w": ones8w,
            "eye81": eye81, "ones128": ones,
        })
    return in_maps


def _combine(parts):
    S = parts.astype(np.float64).sum(axis=0)
    loss_cT = S[COL_AT] - S[COL_BT] + S[COL_CT] / 2 - S[COL_DT] + S[COL_TKT]
    loss_cS = S[COL_AS] - S[COL_BS] + S[COL_CS] / 2 - S[COL_DS] + S[COL_TKS]
    loss_lT = S[COL_LT] - NCORES * NE_CONST
    loss_lS = S[COL_LS] - NCORES * NE_CONST
    N = S[COL_NP]
    return np.array([loss_lT / N, loss_cT / N, loss_lS / N, loss_cS / N],
                    np.float32)


def run_on_hw(inputs, trace=False, **kw):
    nc = _get_nc()
    in_maps = _build_in_maps(inputs)
    res = run_bass_kernel_spmd(nc, in_maps, core_ids=list(range(NCORES)),
                               trace=trace, **kw)
    parts = np.stack([np.asarray(r["out"]).reshape(NPART) for r in res.results])
    return _combine(parts), res


def kernel(**inputs) -> np.ndarray:
    out, _ = run_on_hw(inputs, trace=False)
    return out



# revision 15
# speedup vs baseline: 1.3792x; 1.3792x over previous
"""Trainium2 Bass kernel for nn_AdaptiveMultiBoxLoss (SSD multibox distillation loss).

Data-parallel over the batch dim across 8 NeuronCores; host sums the 8x16
partial columns and performs the final division by N.

v2 design (vs. f32 baseline):
  - host casts conf/loc inputs to bf16: HBM traffic 49 MB -> 26 MB per core
  - Pool engine (gpsimd) builds the one-hot eq (broadcast ops have no DVE
    fast path anyway); PE does the conf[p, ct_p] trace matmul in bf16
  - sumexp per prior via bf16 fold-trees on DVE (2x mode), not tensor_reduce
  - all Ln batched at the end (one act-table load), smooth-L1 in the exact
    0.5*min(u,1)^2 + (u - min(u,1)) form (masked elements contribute 0)
  - unified T/S hard-negative threshold search: both tensors' binary
    searches run lockstep in shared [128, 2*552] ops
  - prior tiling: partition p holds priors [69p, 69p+69); partition 126 is
    end-aligned (priors 8663..8731, first 31 slots masked as duplicates of
    p125's tail), partition 127 zeroed once per buffer slot
"""

import os
import sys

sys.path.insert(0, "/opt/trn_rl_repo")

from contextlib import ExitStack

import ml_dtypes
import numpy as np

import concourse.bass as bass
import concourse.bacc as bacc
import concourse.mybir as mybir
import concourse.tile as tile
from concourse.bass_utils import run_bass_kernel_spmd

F32 = mybir.dt.float32
BF16 = mybir.dt.bfloat16
I32 = mybir.dt.int32
ALU = mybir.AluOpType
ACT = mybir.ActivationFunctionType

# ---- problem geometry (hardcoded) ----
B, P, C = 64, 8732, 81
NCORES = 8
R = B // NCORES            # 8 batch rows per core
NT = 69                    # priors per partition per row (126*69+38 = 8732)
P_FULL = 126 * NT          # 8694 priors on partitions 0..125
TAIL = P - P_FULL          # 38 real priors on partition 126
TAIL_OFF = NT - TAIL       # 31 duplicate slots at the start of p126
RC = R * NT                # 552 row-tiled columns
LTT = 546                  # flat loc tiling: 128 * 546 >= R * P
LF = LTT * 4               # 2184
PADN = 128 * LTT - R * P
NPART = 16
NITER = 7                  # binary search iterations
HI_INIT = 16.0

# partials columns
(COL_BT, COL_BS, COL_AT, COL_AS, COL_CT, COL_CS, COL_DT, COL_DS,
 COL_LT, COL_LS, COL_TKT, COL_TKS, COL_NP) = range(13)

STAGE = int(os.environ.get("K_STAGE", "9"))


def build_nc():
    nc = bacc.Bacc("TRN2", target_bir_lowering=False, debug=False,
                   num_devices=NCORES)

    conf_T = nc.declare_dram_parameter("conf_T", [R, P, C], BF16, isOutput=False)
    conf_S = nc.declare_dram_parameter("conf_S", [R, P, C], BF16, isOutput=False)
    loc_T = nc.declare_dram_parameter("loc_T", [128 * LTT, 4], BF16, isOutput=False)
    loc_S = nc.declare_dram_parameter("loc_S", [128 * LTT, 4], BF16, isOutput=False)
    loc_t = nc.declare_dram_parameter("loc_t", [128 * LTT, 4], BF16, isOutput=False)
    ctbf_p = nc.declare_dram_parameter("ctbf", [128, RC], BF16, isOutput=False)
    ctfl_p = nc.declare_dram_parameter("ctfl", [128, LTT], BF16, isOutput=False)
    eye81_p = nc.declare_dram_parameter("eye81", [81, 81], BF16, isOutput=False)
    eye16_p = nc.declare_dram_parameter("eye16", [16, 16], F32, isOutput=False)
    onesw_p = nc.declare_dram_parameter("onesw16", [16, 128], F32, isOutput=False)
    mask16_p = nc.declare_dram_parameter("mask16", [16, 2], F32, isOutput=False)
    ones_p = nc.declare_dram_parameter("ones128", [128, 1], F32, isOutput=False)
    out_p = nc.declare_dram_parameter("out", [1, NPART], F32, isOutput=True)

    with tile.TileContext(nc) as tc, ExitStack() as ctx:
        cpool = ctx.enter_context(tc.tile_pool(name="consts", bufs=1))
        pers = ctx.enter_context(tc.tile_pool(name="pers", bufs=1))
        small = ctx.enter_context(tc.tile_pool(name="small", bufs=1))
        pool_c = ctx.enter_context(tc.tile_pool(name="conf", bufs=2))
        pool_e = ctx.enter_context(tc.tile_pool(name="expx", bufs=2))
        pool_q = ctx.enter_context(tc.tile_pool(name="eq", bufs=2))
        pool_f = ctx.enter_context(tc.tile_pool(name="fold", bufs=2))
        psum = ctx.enter_context(tc.tile_pool(name="ps", bufs=4, space="PSUM"))
        pstr = ctx.enter_context(tc.tile_pool(name="tr", bufs=1, space="PSUM"))

        # ---- constants ----
        eye81 = cpool.tile([81, 81], BF16)
        eye16 = cpool.tile([16, 16], F32)
        onesw16 = cpool.tile([16, 128], F32)
        mask16 = cpool.tile([16, 2], F32)
        ones128 = cpool.tile([128, 1], F32)
        iota81 = cpool.tile([128, 81], BF16)
        nc.sync.dma_start(out=eye81[:, :], in_=eye81_p.ap())
        nc.sync.dma_start(out=eye16[:, :], in_=eye16_p.ap())
        nc.sync.dma_start(out=onesw16[:, :], in_=onesw_p.ap())
        nc.sync.dma_start(out=mask16[:, :], in_=mask16_p.ap())
        nc.sync.dma_start(out=ones128[:, :], in_=ones_p.ap())
        nc.gpsimd.iota(iota81[:, :], pattern=[[1, 81]], base=0,
                       channel_multiplier=0,
                       allow_small_or_imprecise_dtypes=True)

        # ---- persistent tensors ----
        ctbf = pers.tile([128, RC], BF16)
        posf = pers.tile([128, RC], BF16)
        valid = pers.tile([128, RC], BF16)
        ominus = pers.tile([128, RC], BF16)
        sumexp2 = pers.tile([128, 2, RC], F32)      # becomes lse2 in place
        conf02 = pers.tile([128, 2, RC], F32)
        lcm2 = pers.tile([128, 2, RC], F32)
        sjunk2 = pers.tile([128, 2, RC], F32)
        partials = pers.tile([128, NPART], F32)

        ctflb = pers.tile([128, LTT], BF16)
        posml = pers.tile([128, LTT], BF16)
        posml4 = pers.tile([128, LF], BF16)
        locsb = {n: pers.tile([128, LF], BF16, name=f"loc{n}")
                 for n in ("T", "S", "t")}
        lwu = pers.tile([128, LF], BF16)
        lws = pers.tile([128, LF], BF16)
        lwd = pers.tile([128, LF], BF16)

        nc.gpsimd.memset(partials[:, :], 0.0)

        # ---- conf_t row-tiled (host-prepped bf16, pads = -1) ----
        nc.sync.dma_start(out=ctbf[:, :], in_=ctbf_p.ap())
        nc.vector.tensor_scalar(out=posf[:, :], in0=ctbf[:, :], scalar1=0.5,
                                scalar2=None, op0=ALU.is_gt)
        nc.vector.tensor_scalar(out=valid[:, :], in0=ctbf[:, :], scalar1=-0.5,
                                scalar2=None, op0=ALU.is_gt)
        nc.vector.tensor_tensor(out=ominus[:, :], in0=valid[:, :],
                                in1=posf[:, :], op=ALU.subtract)

        # num_pos per row -> k; duplicated into 16 lanes (T rows 0-7, S 8-15)
        npp = small.tile([128, 16], F32)
        for half in (slice(0, 8), slice(8, 16)):
            nc.vector.tensor_reduce(
                out=npp[:, half],
                in_=posf[:, :].rearrange("p (r t) -> p r t", r=R),
                axis=mybir.AxisListType.X, op=ALU.add)
        ps_np = psum.tile([16, 1], F32, tag="ps")
        nc.tensor.matmul(ps_np[:, :], lhsT=npp[:, :], rhs=ones128[:, :],
                         start=True, stop=True)
        np16 = small.tile([16, 1], F32)
        nc.vector.tensor_copy(out=np16[:, :], in_=ps_np[:, :])
        k16 = small.tile([16, 1], F32)
        nc.vector.tensor_scalar(out=k16[:, :], in0=np16[:, :], scalar1=3.0,
                                scalar2=float(P - 1), op0=ALU.mult, op1=ALU.min)
        nc.vector.tensor_copy(out=partials[0:8, COL_NP:COL_NP + 1],
                              in_=np16[0:8, :])

        # ---- conf_t flat layout (for loc masking; host-prepped bf16) ----
        nc.sync.dma_start(out=ctflb[:, :], in_=ctfl_p.ap())
        nc.vector.tensor_scalar(out=posml[:, :], in0=ctflb[:, :], scalar1=0.5,
                                scalar2=None, op0=ALU.is_gt)

        # ---- loc DMAs ----
        for name, param in (("T", loc_T), ("S", loc_S), ("t", loc_t)):
            nc.sync.dma_start(
                out=locsb[name][:, :],
                in_=param.ap().rearrange("(p j) f -> p (j f)", j=LTT))

        # ---- conf streaming loop ----
        pstr2 = pstr.tile([81, 2, C], F32)
        nmm = 0
        total_mm = R * NT

        def emit_loc_chain():
            # posml4[p, 4j+f] = posml[p, j]
            nc.vector.tensor_copy(
                out=posml4[:, :].rearrange("p (j f) -> p j f", f=4),
                in_=posml[:, :].unsqueeze(2).broadcast_to((128, LTT, 4)))
            for x, col in (("T", COL_LT), ("S", COL_LS)):
                nc.vector.tensor_tensor(out=lwd[:, :], in0=locsb[x][:, :],
                                        in1=locsb["t"][:, :], op=ALU.subtract)
                nc.vector.tensor_tensor(out=lwu[:, :], in0=lwd[:, :],
                                        in1=posml4[:, :], op=ALU.mult)
                nc.scalar.activation(out=lwu[:, :], in_=lwu[:, :], func=ACT.Abs)
                nc.vector.tensor_scalar(out=lws[:, :], in0=lwu[:, :],
                                        scalar1=1.0, scalar2=None, op0=ALU.min)
                # lwd = 0.5 * s^2
                nc.scalar.activation(out=lwd[:, :], in_=lws[:, :],
                                     func=ACT.Square,
                                     scale=float(1.0 / np.sqrt(2.0)))
                nc.vector.tensor_tensor(out=lwu[:, :], in0=lwu[:, :],
                                        in1=lws[:, :], op=ALU.subtract)
                nc.vector.tensor_tensor(out=lwd[:, :], in0=lwd[:, :],
                                        in1=lwu[:, :], op=ALU.add)
                nc.vector.tensor_reduce(out=partials[:, col:col + 1],
                                        in_=lwd[:, :],
                                        axis=mybir.AxisListType.X, op=ALU.add)

        if STAGE >= 2:
            for r in range(R):
                rc = r * NT
                ctile = pool_c.tile([128, 2, NT, C], BF16, name="ctile")
                if r < 2:
                    # p127 never written by DMA: zero once per buffer slot
                    # engines can't address a lone partition 127: zero the
                    # last 32 partitions; the row DMAs then overwrite 96..126
                    nc.gpsimd.memset(ctile[96:128, :, :, :], 0.0)
                for xi, (x, param) in enumerate((("T", conf_T), ("S", conf_S))):
                    rowv = param.ap()[r, :, :]
                    nc.sync.dma_start(
                        out=ctile[0:126, xi, :, :],
                        in_=rowv[0:P_FULL, :].rearrange("(p t) c -> p t c", t=NT))
                    nc.sync.dma_start(
                        out=ctile[126:127, xi, :, :],
                        in_=rowv[P - NT:P, :].unsqueeze(0))

                ex = pool_e.tile([128, 2, NT, C], BF16, name="ex")
                for xi in range(2):
                    nc.scalar.activation(out=ex[:, xi, :, :],
                                         in_=ctile[:, xi, :, :], func=ACT.Exp)

                # one-hot eq (broadcast operands: no DVE fast path exists)
                eq = pool_q.tile([128, NT, C], BF16, name="eq")
                if STAGE >= 3:
                    nc.vector.tensor_tensor(
                        out=eq[:, :, :],
                        in0=iota81[:, :].unsqueeze(1).broadcast_to((128, NT, C)),
                        in1=ctbf[:, rc:rc + NT].unsqueeze(2).broadcast_to(
                            (128, NT, C)),
                        op=ALU.is_equal)

                    # B trace: psum[m, xi, c] += sum_p eq[p,t,m] * conf[p,xi,t,c]
                    for t in range(NT):
                        nc.tensor.matmul(
                            pstr2[:, :, :],
                            lhsT=eq[:, t, :],
                            rhs=ctile[:, :, t, :],
                            start=(nmm == 0), stop=(nmm == total_mm - 1))
                        nmm += 1

                # sumexp fold-tree per tensor (bf16 adds run at 2x on DVE)
                if STAGE >= 4:
                    fold = pool_f.tile([128, 2, NT, 40], BF16, name="fold")
                    for xi in range(2):
                        f = fold[:, xi, :, :]
                        e = ex[:, xi, :, :]
                        nc.vector.tensor_tensor(out=f[:, :, 0:40],
                                                in0=e[:, :, 0:40],
                                                in1=e[:, :, 40:80], op=ALU.add)
                        nc.vector.tensor_tensor(out=f[:, :, 0:20],
                                                in0=f[:, :, 0:20],
                                                in1=f[:, :, 20:40], op=ALU.add)
                        nc.vector.tensor_tensor(out=f[:, :, 0:10],
                                                in0=f[:, :, 0:10],
                                                in1=f[:, :, 10:20], op=ALU.add)
                        nc.vector.tensor_tensor(out=f[:, :, 0:5],
                                                in0=f[:, :, 0:5],
                                                in1=f[:, :, 5:10], op=ALU.add)
                        nc.vector.tensor_tensor(out=f[:, :, 0:2],
                                                in0=f[:, :, 0:2],
                                                in1=f[:, :, 2:4], op=ALU.add)
                        nc.vector.tensor_tensor(out=f[:, :, 0:1],
                                                in0=f[:, :, 0:1],
                                                in1=f[:, :, 1:2], op=ALU.add)
                        se = sumexp2[:, xi, rc:rc + NT]
                        nc.vector.tensor_tensor(out=se, in0=f[:, :, 0],
                                                in1=f[:, :, 4], op=ALU.add)
                        nc.vector.tensor_tensor(out=se, in0=se,
                                                in1=e[:, :, 80], op=ALU.add)
                        nc.vector.tensor_copy(out=conf02[:, xi, rc:rc + NT],
                                              in_=ctile[:, xi, :, 0])

                if r == 1:
                    emit_loc_chain()
        else:
            emit_loc_chain()

        # ---- tail: lse, lcm, epilogue sums ----
        if STAGE >= 5:
            # lse in place of sumexp (one Ln, one table load)
            nc.scalar.activation(out=sumexp2[:, :, :], in_=sumexp2[:, :, :],
                                 func=ACT.Ln)
            lse2 = sumexp2
            nc.vector.tensor_tensor(out=lcm2[:, :, :], in0=lse2[:, :, :],
                                    in1=conf02[:, :, :], op=ALU.subtract)
            nc.vector.tensor_tensor(
                out=lcm2[:, :, :], in0=lcm2[:, :, :],
                in1=ominus[:, :].unsqueeze(1).broadcast_to((128, 2, RC)),
                op=ALU.mult)

            # A = sum(lse*posf), C = sum(conf0*valid), D = sum(conf0*posf)
            for src, mask, (c0, c1) in (
                    (lse2, posf, (COL_AT, COL_AS)),
                    (conf02, valid, (COL_CT, COL_CS)),
                    (conf02, posf, (COL_DT, COL_DS))):
                nc.vector.tensor_tensor(
                    out=sjunk2[:, :, :], in0=src[:, :, :],
                    in1=mask[:, :].unsqueeze(1).broadcast_to((128, 2, RC)),
                    op=ALU.mult)
                assert c1 == c0 + 1
                nc.vector.tensor_reduce(out=partials[:, c0:c1 + 1],
                                        in_=sjunk2[:, :, :],
                                        axis=mybir.AxisListType.X, op=ALU.add)

            # B extraction from the PSUM trace
            nc.vector.tensor_tensor(
                out=sjunk2[0:81, :, 0:81], in0=pstr2[:, :, :],
                in1=eye81[:, :].unsqueeze(1).broadcast_to((81, 2, 81)),
                op=ALU.mult)
            nc.vector.tensor_reduce(out=partials[0:81, COL_BT:COL_BS + 1],
                                    in_=sjunk2[0:81, :, 0:81],
                                    axis=mybir.AxisListType.X, op=ALU.add)

        # ---- unified T/S binary search for per-row top-k thresholds ----
        if STAGE >= 6:
            lo16 = small.tile([16, 1], F32)
            hi16 = small.tile([16, 1], F32)
            tm16 = small.tile([16, 1], F32)
            s16 = small.tile([16, 1], F32)
            ge16 = small.tile([16, 1], I32)
            gei16 = small.tile([16, 1], I32)
            diag16 = small.tile([16, 16], F32)
            trep = small.tile([128, 16], F32)
            cnt16 = small.tile([128, 16], F32)
            ssum16 = small.tile([128, 16], F32)
            ns16 = small.tile([16, 2], F32)
            tk16 = small.tile([16, 1], F32)

            lcm_v = lcm2[:, :, :].rearrange("p x (r t) -> p x r t", r=R)
            sj_v = sjunk2[:, :, :].rearrange("p x (r t) -> p x r t", r=R)

            def bcast16(vec16):
                nc.vector.tensor_tensor(out=diag16[:, :], in0=eye16[:, :],
                                        in1=vec16[:, :].broadcast_to((16, 16)),
                                        op=ALU.mult)
                psA = psum.tile([128, 16], F32, name="psA", tag="ps")
                nc.tensor.matmul(psA[:, :], lhsT=onesw16[:, :],
                                 rhs=diag16[:, :], start=True, stop=True)
                nc.vector.tensor_copy(out=trep[:, :], in_=psA[:, :])

            def counts16(out128x16):
                trv = (trep[:, :].rearrange("p (x r) -> p x r", x=2)
                       .unsqueeze(3).broadcast_to((128, 2, R, NT)))
                nc.vector.tensor_tensor(out=sj_v, in0=lcm_v, in1=trv,
                                        op=ALU.is_gt)
                nc.vector.tensor_reduce(
                    out=out128x16[:, :].rearrange("p (x r) -> p x r", x=2),
                    in_=sj_v, axis=mybir.AxisListType.X, op=ALU.add)

            nc.gpsimd.memset(lo16[:, :], 0.0)
            nc.gpsimd.memset(hi16[:, :], HI_INIT)
            for it in range(NITER):
                nc.vector.tensor_tensor(out=tm16[:, :], in0=lo16[:, :],
                                        in1=hi16[:, :], op=ALU.add)
                nc.vector.tensor_scalar(out=tm16[:, :], in0=tm16[:, :],
                                        scalar1=0.5, scalar2=None, op0=ALU.mult)
                bcast16(tm16)
                counts16(cnt16)
                psB = psum.tile([16, 1], F32, name="psB", tag="ps")
                nc.tensor.matmul(psB[:, :], lhsT=cnt16[:, :],
                                 rhs=ones128[:, :], start=True, stop=True)
                nc.vector.tensor_copy(out=s16[:, :], in_=psB[:, :])
                nc.vector.tensor_tensor(out=ge16[:, :], in0=s16[:, :],
                                        in1=k16[:, :], op=ALU.is_ge)
                nc.vector.copy_predicated(out=lo16[:, :], mask=ge16[:, :],
                                          data=tm16[:, :])
                nc.vector.tensor_scalar(out=gei16[:, :], in0=ge16[:, :],
                                        scalar1=1, scalar2=None,
                                        op0=ALU.bitwise_xor)
                nc.vector.copy_predicated(out=hi16[:, :], mask=gei16[:, :],
                                          data=tm16[:, :])

            # exact pass at t* = lo: topk = ssum + (k - cnt) * lo
            bcast16(lo16)
            counts16(cnt16)
            nc.vector.tensor_tensor(out=sj_v, in0=lcm_v, in1=sj_v, op=ALU.mult)
            nc.vector.tensor_reduce(
                out=ssum16[:, :].rearrange("p (x r) -> p x r", x=2),
                in_=sj_v, axis=mybir.AxisListType.X, op=ALU.add)
            psC = psum.tile([16, 2], F32, name="psC", tag="ps")
            nc.tensor.matmul(psC[:, 0:1], lhsT=cnt16[:, :], rhs=ones128[:, :],
                             start=True, stop=True)
            nc.tensor.matmul(psC[:, 1:2], lhsT=ssum16[:, :], rhs=ones128[:, :],
                             start=True, stop=True)
            nc.vector.tensor_copy(out=ns16[:, :], in_=psC[:, :])
            nc.vector.tensor_tensor(out=tk16[:, :], in0=k16[:, :],
                                    in1=ns16[:, 0:1], op=ALU.subtract)
            nc.vector.tensor_tensor(out=tk16[:, :], in0=tk16[:, :],
                                    in1=lo16[:, :], op=ALU.mult)
            nc.vector.tensor_tensor(out=tk16[:, :], in0=tk16[:, :],
                                    in1=ns16[:, 1:2], op=ALU.add)
            # engine APs must start at a partition multiple of 32: split the
            # 16-lane tk into per-tensor columns via masks instead of slices
            nc.vector.tensor_tensor(out=partials[0:16, COL_TKT:COL_TKT + 1],
                                    in0=tk16[:, :], in1=mask16[:, 0:1],
                                    op=ALU.mult)
            nc.vector.tensor_tensor(out=partials[0:16, COL_TKS:COL_TKS + 1],
                                    in0=tk16[:, :], in1=mask16[:, 1:2],
                                    op=ALU.mult)

        # ---- final partition reduce of partials -> out ----
        psF = psum.tile([1, NPART], F32, name="psF", tag="ps")
        nc.tensor.matmul(psF[:, :], lhsT=ones128[:, :], rhs=partials[:, :],
                         start=True, stop=True)
        fin = small.tile([1, NPART], F32)
        nc.vector.tensor_copy(out=fin[:, :], in_=psF[:, :])
        nc.sync.dma_start(out=out_p.ap(), in_=fin[:, :])
    nc.finalize()
    return nc


_NC_CACHE = None


def _get_nc():
    global _NC_CACHE
    if _NC_CACHE is None:
        _NC_CACHE = build_nc()
    return _NC_CACHE


def _host_consts():
    eye81 = np.eye(81, dtype=ml_dtypes.bfloat16)
    eye16 = np.eye(16, dtype=np.float32)
    onesw16 = np.ones((16, 128), np.float32)
    ones128 = np.ones((128, 1), np.float32)
    mask16 = np.zeros((16, 2), np.float32)
    mask16[0:8, 0] = 1.0
    mask16[8:16, 1] = 1.0
    return eye81, eye16, onesw16, ones128, mask16


def _ct_row_tiled(ct_rows: np.ndarray) -> np.ndarray:
    """[R, P] int -> [128, R*NT] bf16 row-tiled, pads/duplicates = -1."""
    out = np.full((128, RC), -1.0, np.float32)
    for r in range(R):
        out[0:126, r * NT:(r + 1) * NT] = ct_rows[r, 0:P_FULL].reshape(126, NT)
        out[126, r * NT + TAIL_OFF:(r + 1) * NT] = ct_rows[r, P_FULL:P]
    return out.astype(ml_dtypes.bfloat16)


def _ct_flat(ct_rows: np.ndarray) -> np.ndarray:
    flat = np.full(128 * LTT, -1.0, np.float32)
    flat[:R * P] = ct_rows.reshape(-1)
    return flat.reshape(128, LTT).astype(ml_dtypes.bfloat16)


def _build_in_maps(inputs):
    conf_T = np.asarray(inputs["conf_dataT"], np.float32)
    conf_S = np.asarray(inputs["conf_dataS"], np.float32)
    loc_T = np.asarray(inputs["loc_dataT"], np.float32)
    loc_S = np.asarray(inputs["loc_dataS"], np.float32)
    loc_t = np.asarray(inputs["loc_t"], np.float32)
    ct = np.asarray(inputs["conf_t"], np.int32)

    def _padloc(a):
        flat = a.reshape(R * P, 4)
        return np.ascontiguousarray(
            np.pad(flat, ((0, PADN), (0, 0)))).astype(ml_dtypes.bfloat16)

    eye81, eye16, onesw16, ones128, mask16 = _host_consts()
    in_maps = []
    for d in range(NCORES):
        sl = slice(d * R, (d + 1) * R)
        ctsl = ct[sl]
        in_maps.append({
            "conf_T": np.ascontiguousarray(conf_T[sl]).astype(ml_dtypes.bfloat16),
            "conf_S": np.ascontiguousarray(conf_S[sl]).astype(ml_dtypes.bfloat16),
            "loc_T": _padloc(loc_T[sl]), "loc_S": _padloc(loc_S[sl]),
            "loc_t": _padloc(loc_t[sl]),
            "ctbf": _ct_row_tiled(ctsl),
            "ctfl": _ct_flat(ctsl),
            "eye81": eye81, "eye16": eye16,
            "onesw16": onesw16, "ones128": ones128, "mask16": mask16,
        })
    return in_maps


def _combine(parts):
    S = parts.astype(np.float64).sum(axis=0)
    loss_cT = S[COL_AT] - S[COL_BT] + S[COL_CT] - S[COL_DT] + S[COL_TKT]
    loss_cS = S[COL_AS] - S[COL_BS] + S[COL_CS] - S[COL_DS] + S[COL_TKS]
    loss_lT = S[COL_LT]
    loss_lS = S[COL_LS]
    N = S[COL_NP]
    return np.array([loss_lT / N, loss_cT / N, loss_lS / N, loss_cS / N],
                    np.float32)


def run_on_hw(inputs, trace=False, **kw):
    nc = _get_nc()
    in_maps = _build_in_maps(inputs)
    res = run_bass_kernel_spmd(nc, in_maps, core_ids=list(range(NCORES)),
                               trace=trace, **kw)
    parts = np.stack([np.asarray(r["out"]).reshape(NPART) for r in res.results])
    return _combine(parts), res


def kernel(**inputs) -> np.ndarray:
    out, _ = run_on_hw(inputs, trace=False)
    return out
